# revision 98
# baseline (speedup 1.0000x reference)
"""Trainium2 Bass kernel for nn_MixtureOfAdapterWithClassifier.

Strategy: data-parallel over the batch (B=8 -> one batch element per
NeuronCore).  Each core runs LN -> gate -> adapter FFN -> gated combine on
its 1024-token shard with replicated weights.

Speed levers over the f32r baseline (267us -> ~96-99us measured):
  - 16 dummy DoubleRow matmuls on a ones tile (results never read) issued
    at the head of the PE queue: they execute at ~7us while the first
    DMAs land, so the tensor engine reaches its boost pstate before the
    real stream starts (first real matmuls otherwise run 1.5-1.8x slow
    for ~3us).  16 is deliberate -- 24 dummies measurably triggered the
    pod's power/utilization throttle and net-regressed.
  - fp8e4 (e4m3) matmuls in DoubleRow perf mode (2 contraction chunks per
    instruction -> 157 TF/s, 2x bf16; measured 215.5ns per
    [K256, M128, N512] matmul = full rate).  Weights are prescaled
    (x32/x64) on the host so w~N(0,0.02) sits in e4m3's normal range; the
    descale is folded into the relu scale / softmax temperature / combine
    weights.
  - x is uploaded twice: token-major bf16 (LN stats + residual) and
    pre-transposed fp8 (matmul feed).  LayerNorm is folded INTO matmul1 as
    an augmented rank-2 update: per token t, chunk f:
        y1_stored[f,t] = relu( sum_h w1q[h,f] x8[h,t]
                               - cs1[f] m_t + (WS1 b1[f]) s_t )
    where (16m_t, 8s_t) come from on-device bn_stats, transposed to row
    form on the PE (8 tiny transposes), and the correction runs as ONE
    extra fp8 matmul per psum with a zero-padded [128, 128] lhsT -- same
    dtype/mode as the DoubleRow stream, so no PE pipeline flush (a bf16 or
    K=2 aug matmul costs ~2-3x a full DR matmul in stream slots).  The
    per-token 1/(s_t WS1 WS2) descale rides the gated combine weight.
    This removes all 64 PE transposes of xhat and the xhat tensor itself.
  - the gate consumes the raw-x fp8 transpose directly (the reference gate
    runs on raw x, so no LN correction needed), in [D, tokens] orientation
    per quarter: 4 DoubleRow matmuls (lhsT zero-padded to M=128; dual-fp8
    LdWeights rejects M=4) + 1 activation, then per-128-token softmax.
  - host-side algebra (as baseline): LN scale/bias folded into W1/b1,
    adapter dedupe when both domains share LN params, domain mask folded
    into the gate bias, quantized-gate-weight column sums precomputed.
  - scheduling notes (engine queues are in-order; all measured on HW):
    x0/x1 DMA before the xT chunk, LN bn chain emitted before the quarter
    loop, per-quarter msd transposes at the quarter head, w2/xT-q1 DMAs
    deferred past quarter 0's phase A (early HBM bandwidth feeds the
    bn->aug critical path), gate-logit psums in the phase-B pool so the
    quarter-1 transposes never wait on an lg drain, last residual add on
    DVE instead of the slower gpsimd.  Variants that reordered these
    (xT first / LN split per quarter / transposes inside the mm1 stream /
    ps1=4 / small tensors after w1) measured 3-30% SLOWER: a PE-queue
    stall at the head drops the PE out of its boost pstate and slows
    every matmul after it.  Run-to-run variance from pod power throttling
    is +/-5-8%; config choices here were made on multi-run medians.

Numerics (vs fp32 reference, harness metric max|err|/max|expected|):
  measured fp8 path on HW: 1.088e-2  (gate is 2e-2; bf16 fallback ~5e-3
  via mm_mode="bf16")
"""

import sys

for _p in ("/opt/trn_rl_repo", "/root/.axon_site/_ro/trn_rl_repo"):
    if _p not in sys.path:
        sys.path.insert(0, _p)

import ml_dtypes
import numpy as np

B, L, H, F, D = 8, 1024, 1024, 2048, 4
N_CORES = 8
T = (B * L) // N_CORES  # tokens per core
P = 128
HC = H // P  # 8
FC = F // P  # 16
TC = T // P  # 8
TB = 512  # token block (mm1 rhs width == one PSUM bank)
NQ = T // TB  # 2
TCQ = TB // P  # token chunks per quarter
EPS = 1e-6
NEG = -1e9
WS1 = 32.0  # fp8 prescale for w1/gw (keeps relu(y1)*WS1*s below e4m3 max 240)
WS2 = 64.0  # fp8 prescale for w2

MM_DEFAULT = "fp8"

_PROGRAMS = {}


def build_program(n_adapters=1, mm_mode=MM_DEFAULT, has_b2=False):
    import contextlib

    import concourse.bass as bass  # noqa: F401
    import concourse.mybir as mybir
    import concourse.tile as tile
    from concourse import bacc

    dt = mybir.dt
    AF = mybir.ActivationFunctionType
    ALU = mybir.AluOpType

    fp8 = mm_mode == "fp8"
    md = dt.float8e4 if fp8 else dt.bfloat16
    PM = mybir.MatmulPerfMode.DoubleRow if fp8 else None
    ks = 2 if fp8 else 1
    ws1 = WS1 if fp8 else 1.0
    ws2 = WS2 if fp8 else 1.0
    wsg = WS1 if fp8 else 1.0  # gate weight prescale

    nc = bacc.Bacc(
        "TRN2", target_bir_lowering=False, debug=False, num_devices=N_CORES
    )

    x_d = nc.dram_tensor("x", [T, H], dt.bfloat16, kind="ExternalInput").ap()
    # raw x transposed, per-quarter chunks: [q][p(h%128), hc, tokens]
    xt_d = nc.dram_tensor("xT", [NQ, P, HC, TB], md, kind="ExternalInput").ap()
    w1_d = [
        nc.dram_tensor(f"w1_{k}", [P, FC, HC, P], md, kind="ExternalInput").ap()
        for k in range(n_adapters)
    ]
    # aug rows per fc: row0 = -cs1[f]/16, row1 = WS1*b1[f]/8, rows 2..127
    # zero.  Padded to a full 128-K matmul so the aug runs in the same fp8
    # mode as the DoubleRow stream (bf16 aug matmuls cost a pipeline flush).
    a1_d = [
        nc.dram_tensor(f"a1_{k}", [P, FC, P], md, kind="ExternalInput").ap()
        for k in range(n_adapters)
    ]
    w2_d = nc.dram_tensor("w2", [P, FC, H], md, kind="ExternalInput").ap()
    # gate w1 padded to 128 output columns (dual-fp8 LdWeights rejects M=4)
    gw1_d = nc.dram_tensor("gw1", [P, HC, P], md, kind="ExternalInput").ap()
    gw2_d = nc.dram_tensor("gw2", [D, D], md, kind="ExternalInput").ap()
    gb1_d = nc.dram_tensor("gb1c", [D, 1], dt.float32, kind="ExternalInput").ap()
    # gb2b is pre-scaled by wsg on the host (softmax runs at temp 1/wsg)
    gb2_d = nc.dram_tensor("gb2b", [P, D], dt.float32, kind="ExternalInput").ap()
    b2_d = (
        nc.dram_tensor("b2row", [1, H], md, kind="ExternalInput").ap()
        if has_b2
        else None
    )
    out_d = nc.dram_tensor("out", [T, H], dt.bfloat16, kind="ExternalOutput").ap()

    with tile.TileContext(nc) as tc_:
        with contextlib.ExitStack() as ctx:
            singles = ctx.enter_context(tc_.tile_pool(name="singles", bufs=1))
            xpool = ctx.enter_context(tc_.tile_pool(name="xload", bufs=TC))
            spool = ctx.enter_context(tc_.tile_pool(name="stats", bufs=1))
            gpool = ctx.enter_context(tc_.tile_pool(name="gate", bufs=1))
            xqpool = ctx.enter_context(tc_.tile_pool(name="xhT", bufs=2))
            ypool = ctx.enter_context(tc_.tile_pool(name="y1T", bufs=2))
            vpool = ctx.enter_context(tc_.tile_pool(name="comb", bufs=3))
            opool = ctx.enter_context(tc_.tile_pool(name="outb", bufs=4))
            tp_ps = ctx.enter_context(
                tc_.tile_pool(name="tp_ps", bufs=2, space="PSUM")
            )
            gps_ps = ctx.enter_context(
                tc_.tile_pool(name="gps_ps", bufs=1, space="PSUM")
            )
            ps1 = ctx.enter_context(tc_.tile_pool(name="ps1", bufs=3, space="PSUM"))
            ps2 = ctx.enter_context(tc_.tile_pool(name="ps2", bufs=2, space="PSUM"))

            # ---------------- DMA: critical path first ----------------
            # sync ring: xT q0 (mm1+gate feed), x tiles (bn->aug feed), xT q1
            xq_t = []
            for q in range(NQ):
                xq = xqpool.tile([P, HC, TB], md, tag="xq")
                xq_t.append(xq)
            # x0/x1 first: the LN chain feeds the msd transposes at the head
            # of the PE queue; a late bn start stalls the PE out of its
            # boost pstate and slows the whole matmul stream
            x_t = []
            for tci in range(TC):
                xt = xpool.tile([P, H], dt.bfloat16, tag="x")
                x_t.append(xt)
            for tci in range(2):
                nc.sync.dma_start(
                    out=x_t[tci], in_=x_d[tci * P : (tci + 1) * P, :]
                )
            nc.sync.dma_start(out=xq_t[0], in_=xt_d[0])
            for tci in range(2, TC):
                nc.sync.dma_start(
                    out=x_t[tci], in_=x_d[tci * P : (tci + 1) * P, :]
                )
            # xq1 issued later (needed ~45us in) to keep early HBM bandwidth
            # for the x tiles feeding the bn -> aug critical path

            # gpsimd ring: small tensors, then w1 chunks (fc order), then w2
            from concourse.masks import make_identity

            identity_b = singles.tile([P, P], dt.bfloat16, tag="id_b")
            make_identity(nc, identity_b)

            # PE warmup: dummy matmuls (results never read) run while the
            # first DMAs land, so the tensor engine is already at its boost
            # pstate when the real stream starts (first real matmuls
            # otherwise run at 584-667ns vs the steady 379ns)
            warm = singles.tile([P, ks, P], md, tag="warm")
            nc.gpsimd.memset(warm, 1.0)
            wps = gps_ps.tile([P, TB], dt.float32, tag="gps")
            # 16 measured best: more warmup burns power budget and triggers
            # the pod's utilization throttle
            NWARM = 20
            for i in range(NWARM):
                nc.tensor.matmul(
                    wps[:, :P],
                    lhsT=warm,
                    rhs=warm,
                    start=(i == 0),
                    stop=(i == NWARM - 1),
                    perf_mode=PM,
                )

            gw1sb = singles.tile([P, HC, P], md, tag="gw1sb")
            nc.gpsimd.dma_start(out=gw1sb, in_=gw1_d)
            gw2sb = singles.tile([D, D], md, tag="gw2sb")
            nc.gpsimd.dma_start(out=gw2sb, in_=gw2_d)
            gb1c = singles.tile([D, 1], dt.float32, tag="gb1c")
            nc.gpsimd.dma_start(out=gb1c, in_=gb1_d)
            gb2b = singles.tile([P, D], dt.float32, tag="gb2b")
            nc.gpsimd.dma_start(out=gb2b, in_=gb2_d)
            a1sb = []
            for k in range(n_adapters):
                at = singles.tile([P, FC, P], md, tag=f"a1sb{k}")
                nc.gpsimd.dma_start(out=at, in_=a1_d[k])
                a1sb.append(at)
            w1sb = []
            for k in range(n_adapters):
                wt = singles.tile([P, FC, HC, P], md, tag=f"w1sb{k}")
                for fc in range(0, FC, 4):
                    nc.gpsimd.dma_start(
                        out=wt[:, fc : fc + 4, :, :],
                        in_=w1_d[k][:, fc : fc + 4, :, :],
                    )
                w1sb.append(wt)
            # w2 (2MB, first needed at mm2 of quarter 0 ~35us in) is issued
            # after quarter 0's phase A so its transfers don't steal HBM
            # bandwidth from the x tiles during the warmup
            w2sb = singles.tile([P, FC, H], md, tag="w2sb")
            if has_b2:
                b2row = singles.tile([1, H], md, tag="b2row")

            def emit_deferred_loads():
                for fo in range(0, FC, 4):
                    nc.gpsimd.dma_start(
                        out=w2sb[:, fo : fo + 4, :], in_=w2_d[:, fo : fo + 4, :]
                    )
                if has_b2:
                    nc.gpsimd.dma_start(out=b2row, in_=b2_d)
                nc.sync.dma_start(out=xq_t[1], in_=xt_d[1])

            # ---------------- stage 1: LN stats per token chunk ----------
            eps_t = singles.tile([P, 1], dt.float32)
            nc.vector.memset(eps_t, EPS)
            m_t, iv_t, msd_t = [], [], []
            # per-quarter aug rhs [P, TB]: row0 = 16*m_t, row1 = 8*s_t
            # (scaled into e4m3's normal range; host divides the aug lhsT),
            # rows 2..127 zero -> standard full-K fp8 matmul, no mode switch
            augr_q = []
            for q in range(NQ):
                ar = spool.tile([P, TB], md, tag=f"augr{q}")
                nc.gpsimd.memset(ar, 0.0)
                augr_q.append(ar)
            def emit_ln(tci):
                # LN stats chain for one token chunk
                xt = x_t[tci]
                stt = spool.tile([P, 2, 6], dt.float32, tag="st")
                for sg in range(2):
                    nc.vector.bn_stats(
                        out=stt[:, sg, :], in_=xt[:, sg * 512 : (sg + 1) * 512]
                    )
                mv = spool.tile([P, 2], dt.float32, tag=f"mv{tci}")
                nc.vector.bn_aggr(out=mv, in_=stt)
                m = mv[:, 0:1]
                sd = spool.tile([P, 1], dt.float32, tag=f"sd{tci}")
                nc.scalar.activation(
                    out=sd, in_=mv[:, 1:2], func=AF.Sqrt, bias=eps_t, scale=1.0
                )
                iv = spool.tile([P, 1], dt.float32, tag=f"iv{tci}")
                nc.vector.reciprocal(out=iv, in_=sd)
                msd = spool.tile([P, 2], dt.bfloat16, tag=f"msd{tci}")
                nc.vector.tensor_scalar_mul(msd[:, 0:1], m, 16.0)
                nc.scalar.mul(out=msd[:, 1:2], in_=sd, mul=8.0)
                m_t.append(m)
                iv_t.append(iv)
                msd_t.append(msd)

            def emit_msd_transpose(tci):
                q, tcl = tci // TCQ, tci % TCQ
                tps = tp_ps.tile([P, P], dt.bfloat16, tag="tp")
                nc.tensor.transpose(tps[:2, :], msd_t[tci], identity_b)
                nc.vector.tensor_copy(
                    out=augr_q[q][0:2, tcl * P : (tcl + 1) * P], in_=tps[:2, :]
                )

            # ---------------- quarters ----------------
            # LN chains per-quarter: quarter 0's relu drains never queue
            # behind tc4..7's bn/sqrt on DVE/ACT, and the augr copies for
            # q0 ride directly after tc0..3's chains
            for q in range(NQ):
                xq = xq_t[q]
                for tcl in range(TCQ):
                    emit_ln(q * TCQ + tcl)
                    emit_msd_transpose(q * TCQ + tcl)

                # ---- gate: gpsT[d, t] = sum_h gw1q[h,d] x8[h,t] ----
                gps = gps_ps.tile([P, TB], dt.float32, tag="gps")
                for j in range(0, HC, ks):
                    nc.tensor.matmul(
                        gps,
                        lhsT=gw1sb[:, j : j + ks, :],
                        rhs=xq[:, j : j + ks, :],
                        start=(j == 0),
                        stop=(j + ks >= HC),
                        perf_mode=PM,
                    )
                hsT = gpool.tile([D, TB], md, tag="hsT")
                nc.scalar.activation(
                    out=hsT,
                    in_=gps[:D, :],
                    func=AF.Relu,
                    bias=gb1c,
                    scale=1.0 / wsg,
                )

                # ---- phase A: y1T_stored = relu(mm + aug) ----
                y1T = []
                for k in range(n_adapters):
                    yk = ypool.tile([P, FC, TB], md, tag=f"y1T{k}")
                    for fc in range(FC):
                        p1 = ps1.tile([P, TB], dt.float32, tag="ps1")
                        for j in range(0, HC, ks):
                            nc.tensor.matmul(
                                p1,
                                lhsT=w1sb[k][:, fc, j : j + ks, :],
                                rhs=xq[:, j : j + ks, :],
                                start=(j == 0),
                                stop=False,
                                perf_mode=PM,
                            )
                        nc.tensor.matmul(
                            p1,
                            lhsT=a1sb[k][:, fc, :],
                            rhs=augr_q[q],
                            start=False,
                            stop=True,
                        )
                        if fc % 2 == 0:
                            nc.scalar.activation(
                                out=yk[:, fc, :], in_=p1, func=AF.Relu, scale=1.0
                            )
                        else:
                            nc.vector.tensor_scalar_max(yk[:, fc, :], p1, 0.0)
                    y1T.append(yk)

                if q == 0:
                    emit_deferred_loads()

                # ---- gate softmax per token chunk ----
                wa_t = {}
                c0_t = {}
                for tcl in range(TCQ):
                    tci = q * TCQ + tcl
                    # lg psums ride the phase-B pool (fast DVE drains) so
                    # quarter-1's msd transposes never wait on tp_ps
                    lps = ps2.tile([P, TB], dt.float32, tag="ps2")
                    nc.tensor.matmul(
                        lps[:, :D],
                        lhsT=hsT[:, tcl * P : (tcl + 1) * P],
                        rhs=gw2sb,
                        start=True,
                        stop=True,
                    )
                    lg = gpool.tile([P, D], dt.float32, tag="lg")
                    nc.vector.tensor_add(out=lg, in0=lps[:, :D], in1=gb2b)
                    mx = gpool.tile([P, 1], dt.float32, tag="mx")
                    nc.vector.reduce_max(out=mx, in_=lg, axis=mybir.AxisListType.X)
                    nc.scalar.mul(out=mx, in_=mx, mul=-1.0 / wsg)
                    e = gpool.tile([P, D], dt.float32, tag="e")
                    ssum = gpool.tile([P, 1], dt.float32, tag="ss")
                    nc.scalar.activation(
                        out=e,
                        in_=lg,
                        func=AF.Exp,
                        bias=mx,
                        scale=1.0 / wsg,
                        accum_out=ssum,
                    )
                    ivs = gpool.tile([P, 1], dt.float32, tag="ivs")
                    nc.vector.reciprocal(out=ivs, in_=ssum)
                    # combine weight carries the full descale: p/(s*WS1*WS2)
                    ivw = gpool.tile([P, 1], dt.float32, tag="ivw")
                    nc.vector.tensor_scalar(
                        out=ivw,
                        in0=ivs,
                        scalar1=iv_t[tci],
                        scalar2=1.0 / (ws1 * ws2),
                        op0=ALU.mult,
                        op1=ALU.mult,
                    )
                    if n_adapters == 1:
                        t12 = gpool.tile([P, 1], dt.float32, tag="t12")
                        nc.vector.tensor_add(out=t12, in0=e[:, 1:2], in1=e[:, 2:3])
                        wa0 = gpool.tile([P, 1], dt.float32, tag=f"wa0_{tcl}")
                        nc.vector.tensor_mul(out=wa0, in0=t12, in1=ivw)
                        wa_t[(0, tcl)] = wa0
                    else:
                        for k in range(2):
                            wak = gpool.tile([P, 1], dt.float32, tag=f"wa{k}_{tcl}")
                            nc.vector.tensor_mul(
                                out=wak, in0=e[:, 1 + k : 2 + k], in1=ivw
                            )
                            wa_t[(k, tcl)] = wak
                    c0 = gpool.tile([P, 1], dt.float32, tag=f"c0_{tcl}")
                    nc.vector.tensor_mul(out=c0, in0=e[:, 0:1], in1=ivs)
                    nc.scalar.add(out=c0, in_=c0, add=1.0)
                    c0_t[tcl] = c0

                # ---- phase B: y2 psum, combine, store ----
                for tcl in range(TCQ):
                    tci = q * TCQ + tcl
                    for ht in range(H // TB):
                        hsl = slice(ht * TB, (ht + 1) * TB)
                        v = None
                        for k in range(n_adapters):
                            p2 = ps2.tile([P, TB], dt.float32, tag="ps2")
                            for j in range(0, FC, ks):
                                nc.tensor.matmul(
                                    p2,
                                    lhsT=y1T[k][
                                        :, j : j + ks, tcl * P : (tcl + 1) * P
                                    ],
                                    rhs=w2sb[:, j : j + ks, hsl],
                                    start=(j == 0),
                                    stop=(j + ks >= FC and not has_b2),
                                    perf_mode=PM,
                                )
                            if has_b2:
                                # p2 += s_t * (WS1*WS2*b2)[h]; the combine's
                                # 1/(s WS1 WS2) turns this into +b2
                                nc.tensor.matmul(
                                    p2,
                                    lhsT=augr_q[q][1:2, tcl * P : (tcl + 1) * P],
                                    rhs=b2row[:, hsl],
                                    start=False,
                                    stop=True,
                                )
                            vk = vpool.tile([P, TB], dt.float32, tag=f"v{k}")
                            nc.vector.tensor_scalar_mul(vk, p2, wa_t[(k, tcl)])
                            if v is None:
                                v = vk
                            else:
                                nc.vector.tensor_add(out=v, in0=v, in1=vk)
                        xtm = vpool.tile([P, TB], dt.float32, tag="xt")
                        nc.scalar.mul(out=xtm, in_=x_t[tci][:, hsl], mul=c0_t[tcl])
                        ob = opool.tile([P, TB], dt.bfloat16, tag="ob")
                        # last chunk's add on DVE: two serial 1.27us gpsimd
                        # adds would otherwise sit on the critical tail
                        last = q == NQ - 1 and tcl == TCQ - 1
                        (nc.vector if last else nc.gpsimd).tensor_add(
                            out=ob, in0=v, in1=xtm
                        )
                        nc.sync.dma_start(
                            out=out_d[tci * P : (tci + 1) * P, hsl], in_=ob
                        )

    nc.compile()
    return nc


def get_program(n_adapters=1, mm_mode=MM_DEFAULT, has_b2=False):
    key = (n_adapters, mm_mode, has_b2)
    if key not in _PROGRAMS:
        _PROGRAMS[key] = build_program(n_adapters, mm_mode, has_b2)
    return _PROGRAMS[key]


def make_in_maps(inputs, mm_mode=MM_DEFAULT):
    """Host-side prep: fold LN into adapter weights, dedupe adapters, fold
    the domain mask into the gate bias, prescale+cast weights to the matmul
    dtype in SBUF chunk layout, shard x over cores (bf16 + fp8 transpose)."""
    inp = {k: np.asarray(v) for k, v in inputs.items()}
    f32 = np.float32
    fp8 = mm_mode == "fp8"
    md_np = ml_dtypes.float8_e4m3 if fp8 else ml_dtypes.bfloat16
    bf16 = ml_dtypes.bfloat16
    ws1 = WS1 if fp8 else 1.0
    ws2 = WS2 if fp8 else 1.0
    wsg = WS1 if fp8 else 1.0

    x = np.ascontiguousarray(inp["x"], dtype=f32)
    dm = inp["domain_mask"]
    sb, bb = inp["ln_s_book"].astype(f32), inp["ln_b_book"].astype(f32)
    si, bi = inp["ln_s_iwslt"].astype(f32), inp["ln_b_iwslt"].astype(f32)
    w1 = inp["ad_w1"].astype(f32)
    b1 = inp["ad_b1"].astype(f32)

    same = np.array_equal(sb, si) and np.array_equal(bb, bi)
    ln_list = [(sb, bb)] if same else [(sb, bb), (si, bi)]

    folded = []
    for s, b in ln_list:
        w1e = w1 if np.all(s == 1.0) else np.ascontiguousarray(w1 * s[:, None])
        b1e = b1 if not np.any(b) else (b1 + b @ w1).astype(f32)
        folded.append((w1e, b1e))

    gw1 = inp["gate_w1"].astype(f32)
    gw2 = inp["gate_w2"].astype(f32)
    gw1p = np.zeros((H, P), f32)
    gw1p[:, :D] = wsg * gw1
    gw1q = gw1p.astype(md_np)  # [H, 128] zero-padded
    gw2q = (wsg * gw2).astype(md_np)
    gb2e = (
        inp["gate_b2"].astype(f32)
        + np.where(dm == 0, f32(NEG), f32(0.0)).astype(f32)
    )

    b2 = inp["ad_b2"].astype(f32)
    has_b2 = bool(np.any(b2))

    w2q = (ws2 * inp["ad_w2"].astype(f32)).astype(md_np)  # [F, H]
    base = {
        "gw1": np.ascontiguousarray(gw1q.reshape(HC, P, P).transpose(1, 0, 2)),
        "gw2": np.ascontiguousarray(gw2q),
        "gb1c": np.ascontiguousarray(inp["gate_b1"].astype(f32)[:, None]),
        "gb2b": np.broadcast_to((wsg * gb2e).astype(f32), (P, D)).copy(),
        "w2": np.ascontiguousarray(w2q.reshape(FC, P, H).transpose(1, 0, 2)),
    }
    if has_b2:
        # rides the 8*s aug row; combine applies 1/(s*ws1*ws2)
        base["b2row"] = np.ascontiguousarray(
            (ws1 * ws2 / 8.0 * b2).astype(md_np)[None, :]
        )
    for k, (w1e, b1e) in enumerate(folded):
        w1q = (ws1 * w1e).astype(md_np)  # [H, F]
        base[f"w1_{k}"] = np.ascontiguousarray(
            w1q.reshape(HC, P, FC, P).transpose(1, 2, 0, 3)
        )
        # aug rows: [0] = -colsum(w1q)[f], [1] = WS1*b1[f], laid out [2, FC, P]
        cs1 = w1q.astype(f32).sum(0)  # [F]
        # padded fp8 aug lhsT [P, FC, P]: row0 = -cs1/16, row1 = ws1*b1/8
        a1 = np.zeros((P, F), f32)
        a1[0] = -cs1 / 16.0
        a1[1] = ws1 * b1e / 8.0
        base[f"a1_{k}"] = np.ascontiguousarray(
            a1.astype(md_np).reshape(P, FC, P)
        )

    xs = x.reshape(N_CORES, T, H)
    in_maps = []
    for c in range(N_CORES):
        xc = xs[c]
        # [T, H] -> [NQ, P(h%128), HC, TB]
        xT = np.ascontiguousarray(
            xc.reshape(NQ, TB, HC, P).transpose(0, 3, 2, 1).astype(md_np)
        )
        in_maps.append(
            dict(
                base,
                x=np.ascontiguousarray(xc.astype(bf16)),
                xT=xT,
            )
        )
    return in_maps, len(folded), has_b2


def kernel(**inputs):
    from concourse.bass_utils import run_bass_kernel_spmd

    in_maps, n_ad, has_b2 = make_in_maps(inputs, MM_DEFAULT)
    nc = get_program(n_adapters=n_ad, mm_mode=MM_DEFAULT, has_b2=has_b2)
    res = run_bass_kernel_spmd(nc, in_maps, list(range(N_CORES)))
    out = np.stack(
        [
            np.asarray(res.results[c]["out"]).astype(np.float32)
            for c in range(N_CORES)
        ],
        axis=0,
    )
    return out.reshape(B, L, H)


# revision 99
# speedup vs baseline: 1.0832x; 1.0832x over previous
"""Trainium2 Bass kernel for nn_MixtureOfAdapterWithClassifier.

Strategy: data-parallel over the batch (B=8 -> one batch element per
NeuronCore).  Each core runs LN -> gate -> adapter FFN -> gated combine on
its 1024-token shard with replicated weights.

Speed levers over the f32r baseline (267us -> ~96-99us measured):
  - 16 dummy DoubleRow matmuls on a ones tile (results never read) issued
    at the head of the PE queue: they execute at ~7us while the first
    DMAs land, so the tensor engine reaches its boost pstate before the
    real stream starts (first real matmuls otherwise run 1.5-1.8x slow
    for ~3us).  16 is deliberate -- 24 dummies measurably triggered the
    pod's power/utilization throttle and net-regressed.
  - fp8e4 (e4m3) matmuls in DoubleRow perf mode (2 contraction chunks per
    instruction -> 157 TF/s, 2x bf16; measured 215.5ns per
    [K256, M128, N512] matmul = full rate).  Weights are prescaled
    (x32/x64) on the host so w~N(0,0.02) sits in e4m3's normal range; the
    descale is folded into the relu scale / softmax temperature / combine
    weights.
  - x is uploaded twice: token-major bf16 (LN stats + residual) and
    pre-transposed fp8 (matmul feed).  LayerNorm is folded INTO matmul1 as
    an augmented rank-2 update: per token t, chunk f:
        y1_stored[f,t] = relu( sum_h w1q[h,f] x8[h,t]
                               - cs1[f] m_t + (WS1 b1[f]) s_t )
    where (16m_t, 8s_t) come from on-device bn_stats, transposed to row
    form on the PE (8 tiny transposes), and the correction runs as ONE
    extra fp8 matmul per psum with a zero-padded [128, 128] lhsT -- same
    dtype/mode as the DoubleRow stream, so no PE pipeline flush (a bf16 or
    K=2 aug matmul costs ~2-3x a full DR matmul in stream slots).  The
    per-token 1/(s_t WS1 WS2) descale rides the gated combine weight.
    This removes all 64 PE transposes of xhat and the xhat tensor itself.
  - the gate consumes the raw-x fp8 transpose directly (the reference gate
    runs on raw x, so no LN correction needed), in [D, tokens] orientation
    per quarter: 4 DoubleRow matmuls (lhsT zero-padded to M=128; dual-fp8
    LdWeights rejects M=4) + 1 activation, then per-128-token softmax.
  - host-side algebra (as baseline): LN scale/bias folded into W1/b1,
    adapter dedupe when both domains share LN params, domain mask folded
    into the gate bias, quantized-gate-weight column sums precomputed.
  - scheduling notes (engine queues are in-order; all measured on HW):
    x0/x1 DMA before the xT chunk, LN bn chain emitted before the quarter
    loop, per-quarter msd transposes at the quarter head, w2/xT-q1 DMAs
    deferred past quarter 0's phase A (early HBM bandwidth feeds the
    bn->aug critical path), gate-logit psums in the phase-B pool so the
    quarter-1 transposes never wait on an lg drain, last residual add on
    DVE instead of the slower gpsimd.  Variants that reordered these
    (xT first / LN split per quarter / transposes inside the mm1 stream /
    ps1=4 / small tensors after w1) measured 3-30% SLOWER: a PE-queue
    stall at the head drops the PE out of its boost pstate and slows
    every matmul after it.  Run-to-run variance from pod power throttling
    is +/-5-8%; config choices here were made on multi-run medians.

Numerics (vs fp32 reference, harness metric max|err|/max|expected|):
  measured fp8 path on HW: 1.088e-2  (gate is 2e-2; bf16 fallback ~5e-3
  via mm_mode="bf16")
"""

import sys

for _p in ("/opt/trn_rl_repo", "/root/.axon_site/_ro/trn_rl_repo"):
    if _p not in sys.path:
        sys.path.insert(0, _p)

import ml_dtypes
import numpy as np

B, L, H, F, D = 8, 1024, 1024, 2048, 4
N_CORES = 8
T = (B * L) // N_CORES  # tokens per core
P = 128
HC = H // P  # 8
FC = F // P  # 16
TC = T // P  # 8
TB = 512  # token block (mm1 rhs width == one PSUM bank)
NQ = T // TB  # 2
TCQ = TB // P  # token chunks per quarter
EPS = 1e-6
NEG = -1e9
WS1 = 32.0  # fp8 prescale for w1/gw (keeps relu(y1)*WS1*s below e4m3 max 240)
WS2 = 64.0  # fp8 prescale for w2

MM_DEFAULT = "fp8"

_PROGRAMS = {}


def build_program(n_adapters=1, mm_mode=MM_DEFAULT, has_b2=False):
    import contextlib

    import concourse.bass as bass  # noqa: F401
    import concourse.mybir as mybir
    import concourse.tile as tile
    from concourse import bacc

    dt = mybir.dt
    AF = mybir.ActivationFunctionType
    ALU = mybir.AluOpType

    fp8 = mm_mode == "fp8"
    md = dt.float8e4 if fp8 else dt.bfloat16
    PM = mybir.MatmulPerfMode.DoubleRow if fp8 else None
    ks = 2 if fp8 else 1
    ws1 = WS1 if fp8 else 1.0
    ws2 = WS2 if fp8 else 1.0
    wsg = WS1 if fp8 else 1.0  # gate weight prescale

    nc = bacc.Bacc(
        "TRN2", target_bir_lowering=False, debug=False, num_devices=N_CORES
    )

    x_d = nc.dram_tensor("x", [T, H], dt.bfloat16, kind="ExternalInput").ap()
    # raw x transposed, per-quarter chunks: [q][p(h%128), hc, tokens]
    xt_d = nc.dram_tensor("xT", [NQ, P, HC, TB], md, kind="ExternalInput").ap()
    w1_d = [
        nc.dram_tensor(f"w1_{k}", [P, FC, HC, P], md, kind="ExternalInput").ap()
        for k in range(n_adapters)
    ]
    # aug rows per fc: row0 = -cs1[f]/16, row1 = WS1*b1[f]/8, rows 2..127
    # zero.  Padded to a full 128-K matmul so the aug runs in the same fp8
    # mode as the DoubleRow stream (bf16 aug matmuls cost a pipeline flush).
    a1_d = [
        nc.dram_tensor(f"a1_{k}", [P, FC, P], md, kind="ExternalInput").ap()
        for k in range(n_adapters)
    ]
    w2_d = nc.dram_tensor("w2", [P, FC, H], md, kind="ExternalInput").ap()
    # gate w1 padded to 128 output columns (dual-fp8 LdWeights rejects M=4)
    gw1_d = nc.dram_tensor("gw1", [P, HC, P], md, kind="ExternalInput").ap()
    gw2_d = nc.dram_tensor("gw2", [D, D], md, kind="ExternalInput").ap()
    gb1_d = nc.dram_tensor("gb1c", [D, 1], dt.float32, kind="ExternalInput").ap()
    # gb2b is pre-scaled by wsg on the host (softmax runs at temp 1/wsg)
    gb2_d = nc.dram_tensor("gb2b", [P, D], dt.float32, kind="ExternalInput").ap()
    b2_d = (
        nc.dram_tensor("b2row", [1, H], md, kind="ExternalInput").ap()
        if has_b2
        else None
    )
    out_d = nc.dram_tensor("out", [T, H], dt.bfloat16, kind="ExternalOutput").ap()

    with tile.TileContext(nc) as tc_:
        with contextlib.ExitStack() as ctx:
            singles = ctx.enter_context(tc_.tile_pool(name="singles", bufs=1))
            xpool = ctx.enter_context(tc_.tile_pool(name="xload", bufs=TC))
            spool = ctx.enter_context(tc_.tile_pool(name="stats", bufs=1))
            gpool = ctx.enter_context(tc_.tile_pool(name="gate", bufs=1))
            xqpool = ctx.enter_context(tc_.tile_pool(name="xhT", bufs=2))
            ypool = ctx.enter_context(tc_.tile_pool(name="y1T", bufs=2))
            vpool = ctx.enter_context(tc_.tile_pool(name="comb", bufs=3))
            opool = ctx.enter_context(tc_.tile_pool(name="outb", bufs=4))
            tp_ps = ctx.enter_context(
                tc_.tile_pool(name="tp_ps", bufs=2, space="PSUM")
            )
            gps_ps = ctx.enter_context(
                tc_.tile_pool(name="gps_ps", bufs=1, space="PSUM")
            )
            ps1 = ctx.enter_context(tc_.tile_pool(name="ps1", bufs=3, space="PSUM"))
            ps2 = ctx.enter_context(tc_.tile_pool(name="ps2", bufs=2, space="PSUM"))

            # ---------------- DMA: critical path first ----------------
            # sync ring: xT q0 (mm1+gate feed), x tiles (bn->aug feed), xT q1
            xq_t = []
            for q in range(NQ):
                xq = xqpool.tile([P, HC, TB], md, tag="xq")
                xq_t.append(xq)
            # x0/x1 first: the LN chain feeds the msd transposes at the head
            # of the PE queue; a late bn start stalls the PE out of its
            # boost pstate and slows the whole matmul stream
            x_t = []
            for tci in range(TC):
                xt = xpool.tile([P, H], dt.bfloat16, tag="x")
                x_t.append(xt)
            for tci in range(2):
                nc.sync.dma_start(
                    out=x_t[tci], in_=x_d[tci * P : (tci + 1) * P, :]
                )
            nc.sync.dma_start(out=xq_t[0], in_=xt_d[0])
            for tci in range(2, TC):
                nc.sync.dma_start(
                    out=x_t[tci], in_=x_d[tci * P : (tci + 1) * P, :]
                )
            # xq1 issued later (needed ~45us in) to keep early HBM bandwidth
            # for the x tiles feeding the bn -> aug critical path

            # gpsimd ring: small tensors, then w1 chunks (fc order), then w2
            from concourse.masks import make_identity

            identity_b = singles.tile([P, P], dt.bfloat16, tag="id_b")
            make_identity(nc, identity_b)

            # PE warmup: dummy matmuls (results never read) run while the
            # first DMAs land, so the tensor engine is already at its boost
            # pstate when the real stream starts (first real matmuls
            # otherwise run at 584-667ns vs the steady 379ns)
            warm = singles.tile([P, ks, P], md, tag="warm")
            nc.gpsimd.memset(warm, 1.0)
            wps = gps_ps.tile([P, TB], dt.float32, tag="gps")
            # 16 measured best: more warmup burns power budget and triggers
            # the pod's utilization throttle
            NWARM = 16
            for i in range(NWARM):
                nc.tensor.matmul(
                    wps[:, :P],
                    lhsT=warm,
                    rhs=warm,
                    start=(i == 0),
                    stop=(i == NWARM - 1),
                    perf_mode=PM,
                )

            gw1sb = singles.tile([P, HC, P], md, tag="gw1sb")
            nc.gpsimd.dma_start(out=gw1sb, in_=gw1_d)
            gw2sb = singles.tile([D, D], md, tag="gw2sb")
            nc.gpsimd.dma_start(out=gw2sb, in_=gw2_d)
            gb1c = singles.tile([D, 1], dt.float32, tag="gb1c")
            nc.gpsimd.dma_start(out=gb1c, in_=gb1_d)
            gb2b = singles.tile([P, D], dt.float32, tag="gb2b")
            nc.gpsimd.dma_start(out=gb2b, in_=gb2_d)
            a1sb = []
            for k in range(n_adapters):
                at = singles.tile([P, FC, P], md, tag=f"a1sb{k}")
                nc.gpsimd.dma_start(out=at, in_=a1_d[k])
                a1sb.append(at)
            w1sb = []
            for k in range(n_adapters):
                wt = singles.tile([P, FC, HC, P], md, tag=f"w1sb{k}")
                for fc in range(0, FC, 4):
                    nc.gpsimd.dma_start(
                        out=wt[:, fc : fc + 4, :, :],
                        in_=w1_d[k][:, fc : fc + 4, :, :],
                    )
                w1sb.append(wt)
            # w2 (2MB, first needed at mm2 of quarter 0 ~35us in) is issued
            # after quarter 0's phase A so its transfers don't steal HBM
            # bandwidth from the x tiles during the warmup
            w2sb = singles.tile([P, FC, H], md, tag="w2sb")
            if has_b2:
                b2row = singles.tile([1, H], md, tag="b2row")

            def emit_deferred_loads():
                for fo in range(0, FC, 4):
                    nc.gpsimd.dma_start(
                        out=w2sb[:, fo : fo + 4, :], in_=w2_d[:, fo : fo + 4, :]
                    )
                if has_b2:
                    nc.gpsimd.dma_start(out=b2row, in_=b2_d)
                nc.sync.dma_start(out=xq_t[1], in_=xt_d[1])

            # ---------------- stage 1: LN stats per token chunk ----------
            eps_t = singles.tile([P, 1], dt.float32)
            nc.vector.memset(eps_t, EPS)
            m_t, iv_t, msd_t = [], [], []
            # per-quarter aug rhs [P, TB]: row0 = 16*m_t, row1 = 8*s_t
            # (scaled into e4m3's normal range; host divides the aug lhsT),
            # rows 2..127 zero -> standard full-K fp8 matmul, no mode switch
            augr_q = []
            for q in range(NQ):
                ar = spool.tile([P, TB], md, tag=f"augr{q}")
                nc.gpsimd.memset(ar, 0.0)
                augr_q.append(ar)
            def emit_ln(tci):
                # LN stats chain for one token chunk
                xt = x_t[tci]
                stt = spool.tile([P, 2, 6], dt.float32, tag="st")
                for sg in range(2):
                    nc.vector.bn_stats(
                        out=stt[:, sg, :], in_=xt[:, sg * 512 : (sg + 1) * 512]
                    )
                mv = spool.tile([P, 2], dt.float32, tag=f"mv{tci}")
                nc.vector.bn_aggr(out=mv, in_=stt)
                m = mv[:, 0:1]
                sd = spool.tile([P, 1], dt.float32, tag=f"sd{tci}")
                nc.scalar.activation(
                    out=sd, in_=mv[:, 1:2], func=AF.Sqrt, bias=eps_t, scale=1.0
                )
                iv = spool.tile([P, 1], dt.float32, tag=f"iv{tci}")
                nc.vector.reciprocal(out=iv, in_=sd)
                msd = spool.tile([P, 2], dt.bfloat16, tag=f"msd{tci}")
                nc.vector.tensor_scalar_mul(msd[:, 0:1], m, 16.0)
                nc.scalar.mul(out=msd[:, 1:2], in_=sd, mul=8.0)
                m_t.append(m)
                iv_t.append(iv)
                msd_t.append(msd)

            def emit_msd_transpose(tci):
                q, tcl = tci // TCQ, tci % TCQ
                tps = tp_ps.tile([P, P], dt.bfloat16, tag="tp")
                nc.tensor.transpose(tps[:2, :], msd_t[tci], identity_b)
                nc.vector.tensor_copy(
                    out=augr_q[q][0:2, tcl * P : (tcl + 1) * P], in_=tps[:2, :]
                )

            # ---------------- quarters ----------------
            # LN chains per-quarter: quarter 0's relu drains never queue
            # behind tc4..7's bn/sqrt on DVE/ACT, and the augr copies for
            # q0 ride directly after tc0..3's chains
            for q in range(NQ):
                xq = xq_t[q]
                for tcl in range(TCQ):
                    emit_ln(q * TCQ + tcl)
                    emit_msd_transpose(q * TCQ + tcl)

                # ---- gate: gpsT[d, t] = sum_h gw1q[h,d] x8[h,t] ----
                gps = gps_ps.tile([P, TB], dt.float32, tag="gps")
                for j in range(0, HC, ks):
                    nc.tensor.matmul(
                        gps,
                        lhsT=gw1sb[:, j : j + ks, :],
                        rhs=xq[:, j : j + ks, :],
                        start=(j == 0),
                        stop=(j + ks >= HC),
                        perf_mode=PM,
                    )
                hsT = gpool.tile([D, TB], md, tag="hsT")
                nc.scalar.activation(
                    out=hsT,
                    in_=gps[:D, :],
                    func=AF.Relu,
                    bias=gb1c,
                    scale=1.0 / wsg,
                )

                # ---- phase A: y1T_stored = relu(mm + aug) ----
                y1T = []
                for k in range(n_adapters):
                    yk = ypool.tile([P, FC, TB], md, tag=f"y1T{k}")
                    for fc in range(FC):
                        p1 = ps1.tile([P, TB], dt.float32, tag="ps1")
                        for j in range(0, HC, ks):
                            nc.tensor.matmul(
                                p1,
                                lhsT=w1sb[k][:, fc, j : j + ks, :],
                                rhs=xq[:, j : j + ks, :],
                                start=(j == 0),
                                stop=False,
                                perf_mode=PM,
                            )
                        nc.tensor.matmul(
                            p1,
                            lhsT=a1sb[k][:, fc, :],
                            rhs=augr_q[q],
                            start=False,
                            stop=True,
                        )
                        if fc % 2 == 0:
                            nc.scalar.activation(
                                out=yk[:, fc, :], in_=p1, func=AF.Relu, scale=1.0
                            )
                        else:
                            nc.vector.tensor_scalar_max(yk[:, fc, :], p1, 0.0)
                    y1T.append(yk)

                if q == 0:
                    emit_deferred_loads()

                # ---- gate softmax per token chunk ----
                wa_t = {}
                c0_t = {}
                for tcl in range(TCQ):
                    tci = q * TCQ + tcl
                    # lg psums ride the phase-B pool (fast DVE drains) so
                    # quarter-1's msd transposes never wait on tp_ps
                    lps = ps2.tile([P, TB], dt.float32, tag="ps2")
                    nc.tensor.matmul(
                        lps[:, :D],
                        lhsT=hsT[:, tcl * P : (tcl + 1) * P],
                        rhs=gw2sb,
                        start=True,
                        stop=True,
                    )
                    lg = gpool.tile([P, D], dt.float32, tag="lg")
                    nc.vector.tensor_add(out=lg, in0=lps[:, :D], in1=gb2b)
                    mx = gpool.tile([P, 1], dt.float32, tag="mx")
                    nc.vector.reduce_max(out=mx, in_=lg, axis=mybir.AxisListType.X)
                    nc.scalar.mul(out=mx, in_=mx, mul=-1.0 / wsg)
                    e = gpool.tile([P, D], dt.float32, tag="e")
                    ssum = gpool.tile([P, 1], dt.float32, tag="ss")
                    nc.scalar.activation(
                        out=e,
                        in_=lg,
                        func=AF.Exp,
                        bias=mx,
                        scale=1.0 / wsg,
                        accum_out=ssum,
                    )
                    ivs = gpool.tile([P, 1], dt.float32, tag="ivs")
                    nc.vector.reciprocal(out=ivs, in_=ssum)
                    # combine weight carries the full descale: p/(s*WS1*WS2)
                    ivw = gpool.tile([P, 1], dt.float32, tag="ivw")
                    nc.vector.tensor_scalar(
                        out=ivw,
                        in0=ivs,
                        scalar1=iv_t[tci],
                        scalar2=1.0 / (ws1 * ws2),
                        op0=ALU.mult,
                        op1=ALU.mult,
                    )
                    if n_adapters == 1:
                        t12 = gpool.tile([P, 1], dt.float32, tag="t12")
                        nc.vector.tensor_add(out=t12, in0=e[:, 1:2], in1=e[:, 2:3])
                        wa0 = gpool.tile([P, 1], dt.float32, tag=f"wa0_{tcl}")
                        nc.vector.tensor_mul(out=wa0, in0=t12, in1=ivw)
                        wa_t[(0, tcl)] = wa0
                    else:
                        for k in range(2):
                            wak = gpool.tile([P, 1], dt.float32, tag=f"wa{k}_{tcl}")
                            nc.vector.tensor_mul(
                                out=wak, in0=e[:, 1 + k : 2 + k], in1=ivw
                            )
                            wa_t[(k, tcl)] = wak
                    c0 = gpool.tile([P, 1], dt.float32, tag=f"c0_{tcl}")
                    nc.vector.tensor_mul(out=c0, in0=e[:, 0:1], in1=ivs)
                    nc.scalar.add(out=c0, in_=c0, add=1.0)
                    c0_t[tcl] = c0

                # ---- phase B: y2 psum, combine, store ----
                for tcl in range(TCQ):
                    tci = q * TCQ + tcl
                    for ht in range(H // TB):
                        hsl = slice(ht * TB, (ht + 1) * TB)
                        v = None
                        for k in range(n_adapters):
                            p2 = ps2.tile([P, TB], dt.float32, tag="ps2")
                            for j in range(0, FC, ks):
                                nc.tensor.matmul(
                                    p2,
                                    lhsT=y1T[k][
                                        :, j : j + ks, tcl * P : (tcl + 1) * P
                                    ],
                                    rhs=w2sb[:, j : j + ks, hsl],
                                    start=(j == 0),
                                    stop=(j + ks >= FC and not has_b2),
                                    perf_mode=PM,
                                )
                            if has_b2:
                                # p2 += s_t * (WS1*WS2*b2)[h]; the combine's
                                # 1/(s WS1 WS2) turns this into +b2
                                nc.tensor.matmul(
                                    p2,
                                    lhsT=augr_q[q][1:2, tcl * P : (tcl + 1) * P],
                                    rhs=b2row[:, hsl],
                                    start=False,
                                    stop=True,
                                )
                            vk = vpool.tile([P, TB], dt.float32, tag=f"v{k}")
                            nc.vector.tensor_scalar_mul(vk, p2, wa_t[(k, tcl)])
                            if v is None:
                                v = vk
                            else:
                                nc.vector.tensor_add(out=v, in0=v, in1=vk)
                        xtm = vpool.tile([P, TB], dt.float32, tag="xt")
                        nc.scalar.mul(out=xtm, in_=x_t[tci][:, hsl], mul=c0_t[tcl])
                        ob = opool.tile([P, TB], dt.bfloat16, tag="ob")
                        # last chunk's add on DVE: two serial 1.27us gpsimd
                        # adds would otherwise sit on the critical tail
                        last = q == NQ - 1 and tcl == TCQ - 1
                        (nc.vector if last else nc.gpsimd).tensor_add(
                            out=ob, in0=v, in1=xtm
                        )
                        nc.sync.dma_start(
                            out=out_d[tci * P : (tci + 1) * P, hsl], in_=ob
                        )

    nc.compile()
    return nc


def get_program(n_adapters=1, mm_mode=MM_DEFAULT, has_b2=False):
    key = (n_adapters, mm_mode, has_b2)
    if key not in _PROGRAMS:
        _PROGRAMS[key] = build_program(n_adapters, mm_mode, has_b2)
    return _PROGRAMS[key]


def make_in_maps(inputs, mm_mode=MM_DEFAULT):
    """Host-side prep: fold LN into adapter weights, dedupe adapters, fold
    the domain mask into the gate bias, prescale+cast weights to the matmul
    dtype in SBUF chunk layout, shard x over cores (bf16 + fp8 transpose)."""
    inp = {k: np.asarray(v) for k, v in inputs.items()}
    f32 = np.float32
    fp8 = mm_mode == "fp8"
    md_np = ml_dtypes.float8_e4m3 if fp8 else ml_dtypes.bfloat16
    bf16 = ml_dtypes.bfloat16
    ws1 = WS1 if fp8 else 1.0
    ws2 = WS2 if fp8 else 1.0
    wsg = WS1 if fp8 else 1.0

    x = np.ascontiguousarray(inp["x"], dtype=f32)
    dm = inp["domain_mask"]
    sb, bb = inp["ln_s_book"].astype(f32), inp["ln_b_book"].astype(f32)
    si, bi = inp["ln_s_iwslt"].astype(f32), inp["ln_b_iwslt"].astype(f32)
    w1 = inp["ad_w1"].astype(f32)
    b1 = inp["ad_b1"].astype(f32)

    same = np.array_equal(sb, si) and np.array_equal(bb, bi)
    ln_list = [(sb, bb)] if same else [(sb, bb), (si, bi)]

    folded = []
    for s, b in ln_list:
        w1e = w1 if np.all(s == 1.0) else np.ascontiguousarray(w1 * s[:, None])
        b1e = b1 if not np.any(b) else (b1 + b @ w1).astype(f32)
        folded.append((w1e, b1e))

    gw1 = inp["gate_w1"].astype(f32)
    gw2 = inp["gate_w2"].astype(f32)
    gw1p = np.zeros((H, P), f32)
    gw1p[:, :D] = wsg * gw1
    gw1q = gw1p.astype(md_np)  # [H, 128] zero-padded
    gw2q = (wsg * gw2).astype(md_np)
    gb2e = (
        inp["gate_b2"].astype(f32)
        + np.where(dm == 0, f32(NEG), f32(0.0)).astype(f32)
    )

    b2 = inp["ad_b2"].astype(f32)
    has_b2 = bool(np.any(b2))

    w2q = (ws2 * inp["ad_w2"].astype(f32)).astype(md_np)  # [F, H]
    base = {
        "gw1": np.ascontiguousarray(gw1q.reshape(HC, P, P).transpose(1, 0, 2)),
        "gw2": np.ascontiguousarray(gw2q),
        "gb1c": np.ascontiguousarray(inp["gate_b1"].astype(f32)[:, None]),
        "gb2b": np.broadcast_to((wsg * gb2e).astype(f32), (P, D)).copy(),
        "w2": np.ascontiguousarray(w2q.reshape(FC, P, H).transpose(1, 0, 2)),
    }
    if has_b2:
        # rides the 8*s aug row; combine applies 1/(s*ws1*ws2)
        base["b2row"] = np.ascontiguousarray(
            (ws1 * ws2 / 8.0 * b2).astype(md_np)[None, :]
        )
    for k, (w1e, b1e) in enumerate(folded):
        w1q = (ws1 * w1e).astype(md_np)  # [H, F]
        base[f"w1_{k}"] = np.ascontiguousarray(
            w1q.reshape(HC, P, FC, P).transpose(1, 2, 0, 3)
        )
        # aug rows: [0] = -colsum(w1q)[f], [1] = WS1*b1[f], laid out [2, FC, P]
        cs1 = w1q.astype(f32).sum(0)  # [F]
        # padded fp8 aug lhsT [P, FC, P]: row0 = -cs1/16, row1 = ws1*b1/8
        a1 = np.zeros((P, F), f32)
        a1[0] = -cs1 / 16.0
        a1[1] = ws1 * b1e / 8.0
        base[f"a1_{k}"] = np.ascontiguousarray(
            a1.astype(md_np).reshape(P, FC, P)
        )

    xs = x.reshape(N_CORES, T, H)
    in_maps = []
    for c in range(N_CORES):
        xc = xs[c]
        # [T, H] -> [NQ, P(h%128), HC, TB]
        xT = np.ascontiguousarray(
            xc.reshape(NQ, TB, HC, P).transpose(0, 3, 2, 1).astype(md_np)
        )
        in_maps.append(
            dict(
                base,
                x=np.ascontiguousarray(xc.astype(bf16)),
                xT=xT,
            )
        )
    return in_maps, len(folded), has_b2


def kernel(**inputs):
    from concourse.bass_utils import run_bass_kernel_spmd

    in_maps, n_ad, has_b2 = make_in_maps(inputs, MM_DEFAULT)
    nc = get_program(n_adapters=n_ad, mm_mode=MM_DEFAULT, has_b2=has_b2)
    res = run_bass_kernel_spmd(nc, in_maps, list(range(N_CORES)))
    out = np.stack(
        [
            np.asarray(res.results[c]["out"]).astype(np.float32)
            for c in range(N_CORES)
        ],
        axis=0,
    )
    return out.reshape(B, L, H)
